# revision 27
# baseline (speedup 1.0000x reference)
"""Bass/Tile TRN2 kernel for nn_Attention (B=32, S=2048, D=1024), 8 cores.

Algorithm (algebraically equal to the reference, verified to ~6e-6 rel err):
    v[b,:]  = st[b] @ W                  (tiny matmul; avoids the huge hx@W^T)
    c[b]    = st[b] . b
    score   = (hx . v + c) * (mask + 1e-18)
    e       = exp(score - max); L = sum(e)
    u[b,:]  = e . hx                     (PE matmul, hx in native layout)
    ct      = (u @ W^T)/L + b            (softmax sums to 1)

Data-parallel over batch: each of the 8 cores gets 4 batches; W/b replicated.
hx is streamed from HBM exactly once (32MB/core) -> memory-bound kernel.
"""

import numpy as np
from contextlib import ExitStack

import concourse.bass as bass
import concourse.bacc as bacc
import concourse.bass_isa as bass_isa
import concourse.mybir as mybir
import concourse.tile as tile
from concourse.bass_utils import run_bass_kernel_spmd

B, S, D = 32, 2048, 1024
NCORES = 8
BPC = B // NCORES          # 4 batches per core
P = 128
NT = S // P                # 16 s-tiles of 128 tokens per batch
NH = 8                     # stream each batch's hx in 8 slices
TPH = NT // NH             # 8 s-tiles per half
DCH = D // P               # 8 chunks of 128 along D
HF = 512                   # fp32 moving-operand limit per matmul

F32 = mybir.dt.float32
F32R = mybir.dt.float32r
BF16 = mybir.dt.bfloat16
AF = mybir.ActivationFunctionType
ALU = mybir.AluOpType
EPS = 1e-18


def build_nc() -> bass.Bass:
    nc = bacc.Bacc("TRN2", target_bir_lowering=False, debug=False)
    st_d = nc.declare_dram_parameter("st", [BPC, D], F32, isOutput=False)
    hx_d = nc.declare_dram_parameter("hx", [BPC, S, D], F32, isOutput=False)
    hm_d = nc.declare_dram_parameter("hx_mask", [BPC, S], F32, isOutput=False)
    w_d = nc.declare_dram_parameter("W", [D, D], F32, isOutput=False)
    bv_d = nc.declare_dram_parameter("b", [D], F32, isOutput=False)
    id_d = nc.declare_dram_parameter("ident", [P, P], F32, isOutput=False)
    ct_d = nc.declare_dram_parameter("ct", [BPC, D], F32, isOutput=True)

    with tile.TileContext(nc) as tc, ExitStack() as ctx:
        const = ctx.enter_context(tc.tile_pool(name="const", bufs=1))
        wtp = ctx.enter_context(tc.tile_pool(name="wtp", bufs=1))
        wnatp = ctx.enter_context(tc.tile_pool(name="wnatp", bufs=1))
        hxp = ctx.enter_context(tc.tile_pool(name="hxp", bufs=14))
        scrp = ctx.enter_context(tc.tile_pool(name="scrp", bufs=1))
        smp = ctx.enter_context(tc.tile_pool(name="smp", bufs=2))
        vbp = ctx.enter_context(tc.tile_pool(name="vbp", bufs=2))
        psp = ctx.enter_context(tc.tile_pool(name="psp", bufs=2, space="PSUM"))

        # ---- constants / small inputs ----
        ident = const.tile([P, P], F32, name="ident_sb")
        nc.scalar.dma_start(out=ident[:, :], in_=id_d[:, :])
        ident_r = const.tile([P, P], F32R, name="ident_r")
        nc.scalar.dma_start(out=ident_r[:, :], in_=id_d[:, :].bitcast(F32R))
        st_sb = const.tile([BPC, D], F32, name="st_sb")
        nc.scalar.dma_start(out=st_sb[:, :], in_=st_d[:, :])
        bias_row = const.tile([1, D], F32, name="bias_row")
        nc.scalar.dma_start(out=bias_row[:, :], in_=bv_d[None, :])
        mask_nat = const.tile([BPC * NT, P], F32, name="mask_nat")
        nc.scalar.dma_start(
            out=mask_nat[:, :], in_=hm_d.rearrange("b (i p) -> (b i) p", p=P)
        )
        ones14 = const.tile([1, BPC], F32, name="ones14")
        nc.vector.memset(ones14[:, :], 1.0)

        # ---- transpose st -> stT[128e, 4b] chunks ----
        stT = const.tile([P, BPC * DCH], F32R, name="stT")
        for j in range(DCH):
            tp = psp.tile([P, P], F32, name=f"tp_st{j}", tag="tp")
            nc.tensor.transpose(
                tp[:, 0:BPC], st_sb[0:BPC, j * P:(j + 1) * P], ident[0:BPC, 0:BPC]
            )
            nc.scalar.copy(stT[:, j * BPC:(j + 1) * BPC], tp[:, 0:BPC])

        # ---- mask -> [128, 4*16] (partition = s%128, col = b*16 + s//128) ----
        tpm = psp.tile([P, P], F32, name="tpm", tag="tp")
        nc.tensor.transpose(
            tpm[:, 0:BPC * NT],
            mask_nat[0:BPC * NT, :],
            ident[0:BPC * NT, 0:BPC * NT],
        )
        mask1 = const.tile([P, BPC * NT], F32, name="mask1")
        # mask + (1-mask)*1e-18 == mask + 1e-18 exactly in fp32
        nc.vector.tensor_scalar_add(mask1[:, :], tpm[:, 0:BPC * NT], EPS)

        # ---- v = st@W, c = st.b; W^T built on the fly ----
        wt_tiles = [
            wtp.tile([P, D], BF16, name=f"wt{i}", tag=f"wt{i}") for i in range(DCH)
        ]
        v_ps = [
            psp.tile([BPC, HF], F32, name=f"v_ps{h}", tag=f"v{h}", bufs=1) for h in range(2)
        ]
        wn_all = wnatp.tile([P, DCH * D], F32R, name="wn_all", tag="wn")
        wn_tiles = [wn_all[:, j * D:(j + 1) * D] for j in range(DCH)]
        for j in range(DCH):
            wn = wn_tiles[j]
            nc.sync.dma_start(out=wn[:, :], in_=w_d[j * P:(j + 1) * P, :].bitcast(F32R))
            lt = stT[:, j * BPC:(j + 1) * BPC]
            for h in range(2):
                nc.tensor.matmul(
                    v_ps[h][:, :], lt, wn[:, h * HF:(h + 1) * HF],
                    start=(j == 0), stop=(j == DCH - 1),
                )

        v_sb = const.tile([BPC, D], F32, name="v_sb")
        for h in range(2):
            nc.scalar.copy(v_sb[:, h * HF:(h + 1) * HF], v_ps[h][:, :])
        bias4 = const.tile([BPC, D], F32, name="bias4")
        nc.gpsimd.partition_broadcast(bias4[:, :], bias_row[0:1, :])
        c_scr = const.tile([BPC, D], F32, name="c_scr")
        c_sb = const.tile([BPC, 1], F32, name="c_sb")
        nc.vector.scalar_tensor_tensor(
            out=c_scr[:, :], in0=st_sb[:, :], scalar=1.0, in1=bias4[:, :],
            op0=ALU.mult, op1=ALU.mult, accum_out=c_sb[:, 0:1],
        )
        tpc = psp.tile([P, P], F32, name="tpc", tag="tp")
        nc.tensor.transpose(tpc[0:1, 0:BPC], c_sb[:, :], ident[0:BPC, 0:BPC])
        c_row = const.tile([1, BPC], F32, name="c_row")
        nc.scalar.copy(c_row[:, :], tpc[0:1, 0:BPC])
        c_bcast = const.tile([P, BPC], F32, name="c_bcast")
        nc.gpsimd.partition_broadcast(c_bcast[:, :], c_row[0:1, :])
        vb_tiles = {}
        for b in range(BPC):
            v_row = smp.tile([1, D], F32, name=f"v_row{b}", tag="v_row", bufs=1)
            nc.scalar.dma_start(out=v_row[:, :], in_=v_sb[b:b + 1, :])
            vb = vbp.tile([P, D], F32, name=f"vb{b}", tag="vb")
            nc.gpsimd.partition_broadcast(vb[:, :], v_row[0:1, :])
            vb_tiles[b] = vb
        for j in range(DCH):
            for i in range(DCH):
                tp = psp.tile([P, P], F32R, name=f"tp_w{j}_{i}", tag="tp")
                nc.tensor.transpose(
                    tp[:, :], wn_tiles[j][:, i * P:(i + 1) * P], ident_r[:, :]
                )
                nc.scalar.copy(wt_tiles[i][:, j * P:(j + 1) * P], tp[:, :])

        # ---- per-batch streaming ----
        ut_tiles = [
            const.tile([P, BPC], BF16, name=f"ut{k}") for k in range(DCH)
        ]
        for b in range(BPC):
            vb = vb_tiles[b]

            score = smp.tile([P, NT], F32, name=f"score{b}", tag="score")
            hx_half = []
            for h in range(NH):
                hxt = hxp.tile([P, TPH * D], F32R, name=f"hx{b}_{h}", tag="hx")
                nc.sync.dma_start(
                    out=hxt[:, :].rearrange("p (i d) -> p i d", d=D),
                    in_=hx_d[b, h * TPH * P:(h + 1) * TPH * P, :].rearrange(
                        "(i p) d -> p i d", p=P
                    ).bitcast(F32R),
                )
                hx_half.append(hxt)
                warm = psp.tile([1, HF], F32, name=f"warm{b}_{h}", tag="tp")
                nc.tensor.matmul(
                    warm[:, :], hxt[:, 0:1], hxt[:, 0:HF], start=True, stop=True,
                )
                for i2 in range(TPH):
                    i = h * TPH + i2
                    scr = scrp.tile([P, D], F32, name=f"scr{b}_{i}", tag="scr")
                    nc.vector.scalar_tensor_tensor(
                        out=scr[:, :],
                        in0=hxt[:, i2 * D:(i2 + 1) * D].bitcast(F32),
                        scalar=1.0,
                        in1=vb[:, :],
                        op0=ALU.mult,
                        op1=ALU.mult,
                        accum_out=score[:, i:i + 1],
                    )

            score_m = smp.tile([P, NT], F32, name=f"score_m{b}", tag="score_m")
            nc.vector.scalar_tensor_tensor(
                out=score_m[:, :], in0=score[:, :], scalar=c_bcast[:, b:b + 1],
                in1=mask1[:, b * NT:(b + 1) * NT], op0=ALU.add, op1=ALU.mult,
            )
            m1 = smp.tile([P, 1], F32, name=f"m1_{b}", tag="m1")
            nc.vector.tensor_reduce(m1[:, :], score_m[:, :], mybir.AxisListType.X, ALU.max)
            mb = smp.tile([P, 1], F32, name=f"mb_{b}", tag="mb")
            nc.gpsimd.partition_all_reduce(mb[:, :], m1[:, :], P, bass_isa.ReduceOp.max)
            neg_m = smp.tile([P, 1], F32, name=f"negm_{b}", tag="negm")
            nc.vector.tensor_scalar_mul(neg_m[:, :], mb[:, :], -1.0)

            e_sb = smp.tile([P, NT], F32R, name=f"e{b}", tag="e")
            l1 = smp.tile([P, 1], F32, name=f"l1_{b}", tag="l1")
            nc.scalar.activation(
                e_sb[:, :], score_m[:, :], AF.Exp,
                bias=neg_m[:, 0:1], scale=1.0, accum_out=l1[:, 0:1],
            )
            lb = smp.tile([P, 1], F32, name=f"lb_{b}", tag="lb")
            nc.gpsimd.partition_all_reduce(lb[:, :], l1[:, :], P, bass_isa.ReduceOp.add)
            recip_l = smp.tile([P, 1], F32, name=f"recipl_{b}", tag="recipl")
            nc.vector.reciprocal(recip_l[:, :], lb[:, :])

            u_ps = [
                psp.tile([1, HF], F32, name=f"u_ps{b}_{h}", tag=f"u{h}", bufs=1)
                for h in range(2)
            ]
            for i in range(NT):
                h, i2 = divmod(i, TPH)  # quarter h, tile-in-quarter i2
                for hf in range(2):
                    nc.tensor.matmul(
                        u_ps[hf][:, :],
                        e_sb[:, i:i + 1],
                        hx_half[h][:, i2 * D + hf * HF:i2 * D + (hf + 1) * HF],
                        start=(i == 0), stop=(i == NT - 1),
                    )
            uhat = smp.tile([1, D], F32, name=f"uhat{b}", tag="uhat", bufs=1)
            for hf in range(2):
                nc.scalar.mul(
                    uhat[:, hf * HF:(hf + 1) * HF], u_ps[hf][:, :],
                    mul=recip_l[0:1, 0:1],
                )
            for k in range(DCH):
                tp = psp.tile([P, P], F32, name=f"tp_u{b}_{k}", tag="tp")
                nc.tensor.transpose(
                    tp[:, 0:1], uhat[0:1, k * P:(k + 1) * P], ident[0:1, 0:1]
                )
                nc.scalar.copy(ut_tiles[k][:, b:b + 1], tp[:, 0:1])
            ct_row = smp.tile([1, D], F32, name=f"ct_row{b}", tag="ct_row", bufs=1)
            for hf in range(2):
                ctp = psp.tile([1, HF], F32, name=f"ct_ps{b}_{hf}", tag=f"v{hf}",
                               bufs=1)
                for k in range(DCH):
                    nc.tensor.matmul(
                        ctp[:, :], ut_tiles[k][:, b:b + 1],
                        wt_tiles[k][:, hf * HF:(hf + 1) * HF],
                        start=(k == 0), stop=False,
                    )
                nc.tensor.matmul(
                    ctp[:, :], ones14[:, 0:1], bias_row[:, hf * HF:(hf + 1) * HF],
                    start=False, stop=True,
                )
                nc.scalar.copy(ct_row[:, hf * HF:(hf + 1) * HF], ctp[:, :])
            nc.scalar.dma_start(out=ct_d[b:b + 1, :], in_=ct_row[:, :])


    nc.compile()
    return nc


_NC_CACHE = {}


def get_nc() -> bass.Bass:
    if "nc" not in _NC_CACHE:
        _NC_CACHE["nc"] = build_nc()
    return _NC_CACHE["nc"]


def kernel(st, hx, hx_mask, W, b):
    nc = get_nc()
    ident = np.eye(P, dtype=np.float32)
    in_maps = []
    for i in range(NCORES):
        sl = slice(i * BPC, (i + 1) * BPC)
        in_maps.append(
            {
                "st": np.ascontiguousarray(st[sl], dtype=np.float32),
                "hx": np.ascontiguousarray(hx[sl], dtype=np.float32),
                "hx_mask": np.ascontiguousarray(hx_mask[sl], dtype=np.float32),
                "W": np.asarray(W, dtype=np.float32),
                "b": np.asarray(b, dtype=np.float32),
                "ident": ident,
            }
        )
    res = run_bass_kernel_spmd(nc, in_maps, list(range(NCORES)))
    out = np.concatenate([res.results[i]["ct"] for i in range(NCORES)], axis=0)
    return out.astype(np.float32)
